# revision 30
# baseline (speedup 1.0000x reference)
import sys

sys.path.insert(0, "/opt/trn_rl_repo")

import numpy as np  # noqa: E402
import ml_dtypes  # noqa: E402

import bass_rust as _br  # noqa: E402
import concourse.bass as bass  # noqa: E402
import concourse.mybir as mybir  # noqa: E402
import concourse.tile as tile  # noqa: E402
from contextlib import ExitStack  # noqa: E402
from concourse import bacc  # noqa: E402
from concourse.bass_utils import run_bass_kernel_spmd  # noqa: E402

F32 = mybir.dt.float32
BF16 = mybir.dt.bfloat16
F8 = mybir.dt.float8e4
DR = mybir.MatmulPerfMode.DoubleRow
AF = mybir.ActivationFunctionType
ALU = mybir.AluOpType
AX = mybir.AxisListType
BFNP = ml_dtypes.bfloat16
F8NP = mybir.dt.np(mybir.dt.float8e4)

S = 4  # samples per core
C, H, W = 256, 28, 28
N = H * W  # 784
NK = 196
HEADS, DK = 8, 32
CM = 1024
SCALE = DK ** -0.5
EPS = 1e-5
INV_NTOT = 1.0 / (C * N)
ISL = [(0, 512), (512, 272)]  # bank-aligned free splits of 784
NCORES = 8

_CACHE = {}


def _build():
    if "nc" in _CACHE:
        return _CACHE["nc"]
    nc = bacc.Bacc()

    x_d = nc.dram_tensor("x", [S, C, H, W], BF16, kind="ExternalInput")
    y_d = nc.dram_tensor("y", [S, C, H, W], F32, kind="ExternalOutput")

    def din(name, shape, dt=BF16):
        return nc.dram_tensor(name, shape, dt, kind="ExternalInput")

    # host-prepacked weights, all in final SBUF layout [128, cols]
    wpk_d = din("wpk", [128, 2048])        # q/k/v/o: (w*2+kc)*256 + m
    c1t_d = din("c1t", [128, 2048], F8)    # mc*256 + kc*128 + m
    c2t_d = din("c2t", [128, 2048], F8)    # p*512 + mc*256 + t*128 + m
    e8_d = din("e8", [128, 8 * 1568])      # exp(pos)^T per head
    dgl_d = din("dgl", [128, 18 * 128], F8)  # lpu diag, DoubleRow pairs
    dgk_d = din("dgk", [128, 8 * 128])     # kv diag: (ch*4+t)*128
    dgd_d = din("dgd", [128, 72 * 128], F8)  # dw2 diag, DoubleRow pairs
    bcol_d = din("bcol", [128, 48], F32)   # packed bias/fold columns
    brow_d = din("brow", [1, 384])         # bv row (256) + ones row (128)
    bh4_d = din("bh4", [128, 1024], F8)    # head-sum masks, A/B pairs
    idn_d = din("idn", [128, 128])         # identity (pos-bias add matmuls)

    scr_d = nc.dram_tensor("scr", [S, N * C], BF16)
    xv = x_d.rearrange("s c h w -> s c (h w)")
    yv = y_d.rearrange("s c h w -> s c (h w)")

    with tile.TileContext(nc) as tc, ExitStack() as stk:
        cst = stk.enter_context(tc.tile_pool(name="cst", bufs=1))
        wk = stk.enter_context(tc.tile_pool(name="wk", bufs=2))
        psB = stk.enter_context(tc.tile_pool(name="psB", bufs=3, space="PSUM"))
        psS = stk.enter_context(tc.tile_pool(name="psS", bufs=2, space="PSUM"))

        # ---------- boot: criticals + ALL input tiles first on the sync
        # ring, so the bulk weight loads (queued after on the same ring)
        # cannot starve them at the SDMA engines ----
        dgl_sb = cst.tile([128, 18 * 128], F8, tag="dgl_sb")
        nc.sync.dma_start(out=dgl_sb, in_=dgl_d[:, :])
        bcol = cst.tile([128, 48], F32, tag="bcol")
        nc.sync.dma_start(out=bcol, in_=bcol_d[:, :])
        xst = []
        for s in range(S):
            row = []
            for ch in range(2):
                xs = cst.tile([128, N], BF16, tag=f"xs{s}{ch}", name="xs")
                nc.sync.dma_start(
                    out=xs, in_=xv[s, ch * 128:(ch + 1) * 128, :])
                row.append(xs)
            xst.append(row)
        dgk_sb = cst.tile([128, 8 * 128], BF16, tag="dgk_sb")
        nc.scalar.dma_start(out=dgk_sb, in_=dgk_d[:, :])
        wsb = cst.tile([128, 2048], BF16, tag="wsb")
        nc.scalar.dma_start(out=wsb, in_=wpk_d[:, :])
        brow = cst.tile([1, 384], BF16, tag="brow")
        nc.scalar.dma_start(out=brow, in_=brow_d[:, :])
        bh = cst.tile([128, 1024], F8, tag="bh")
        nc.scalar.dma_start(out=bh, in_=bh4_d[:, :])
        idn = cst.tile([128, 128], BF16, tag="idn")
        nc.scalar.dma_start(out=idn, in_=idn_d[:, :])
        onesM = cst.tile([128, 128], F32, tag="onesM")
        nc.vector.memset(onesM, INV_NTOT)

        def strided(ap, pattern):
            c = ap.copy()
            c.ap = _br.VecI64Pair(pattern)
            return c
        # bulk loads: same sync ring, strictly after the input DMAs
        e8 = cst.tile([128, 8 * 1568], BF16, tag="e8")
        nc.sync.dma_start(out=e8, in_=e8_d[:, :])
        c1t = cst.tile([128, 2048], F8, tag="c1t")
        nc.sync.dma_start(out=c1t, in_=c1t_d[:, :])
        c2t = cst.tile([128, 2048], F8, tag="c2t")
        nc.sync.dma_start(out=c2t, in_=c2t_d[:, :])
        dgd_sb = cst.tile([128, 72 * 128], F8, tag="dgd_sb")
        nc.sync.dma_start(out=dgd_sb, in_=dgd_d[:, :])

        # bias/fold column APs
        lpub = [bcol[:, 0:1], bcol[:, 1:2]]
        dwb = [bcol[:, 2:3], bcol[:, 3:4]]
        bqc = [bcol[:, 4:5], bcol[:, 5:6]]
        bkc = [bcol[:, 6:7], bcol[:, 7:8]]
        boc = [bcol[:, 8:9], bcol[:, 9:10]]
        epsc = bcol[:, 10:11]
        A1c = [bcol[:, 12 + i:13 + i] for i in range(8)]
        B1c = [bcol[:, 20 + i:21 + i] for i in range(8)]
        A2c = [bcol[:, 28 + i:29 + i] for i in range(8)]
        B2c = [bcol[:, 36 + i:37 + i] for i in range(8)]
        A3c = [bcol[:, 44 + i:45 + i] for i in range(2)]
        B3c = [bcol[:, 46 + i:47 + i] for i in range(2)]
        bv_r = brow[0:1, 0:256]
        ones1 = brow[0:1, 256:384]

        # LN over (C,H,W): returns (mean, rstd) [128,1] APs.
        # rsqrt via DVE-only Newton (seed 1/v) so ScalarE never needs the
        # Ln table set — Ln and Exp live in different table sets and each
        # Ln would cost two 1.3us table reloads. Converges for v > 1/3;
        # LN variance here is ~1 by construction.
        def ln_stats(chunks, st4, nsum, tagp):
            # st4 cols [0:nsum) hold per-chunk partial sums, accumulated for
            # free by the producers' accum_out; we add the two sum-of-squares
            # cols here and do the cross-partition reduction with one matmul.
            for ch in range(2):
                sq = wk.tile([128, N], BF16, tag="sqscr", bufs=2, name="sq")
                nc.scalar.activation(
                    out=sq, in_=chunks[ch], func=AF.Square,
                    accum_out=st4[:, nsum + ch:nsum + ch + 1])
            pst = psS.tile([128, nsum + 2], F32, tag="small", name="pst")
            nc.tensor.matmul(pst, onesM, st4, start=True, stop=True)
            stc = wk.tile([128, nsum + 2], F32, tag=f"stc{tagp}", bufs=2,
                          name="stc")
            nc.vector.tensor_copy(out=stc, in_=pst)
            mean = wk.tile([128, 1], F32, tag=f"mean{tagp}", bufs=2,
                           name="mean")
            if nsum == 4:
                u2 = wk.tile([128, 2], F32, tag=f"u2{tagp}", bufs=2, name="u2")
                nc.vector.tensor_add(out=u2, in0=stc[:, 0:2], in1=stc[:, 2:4])
                nc.vector.tensor_add(out=mean, in0=u2[:, 0:1], in1=u2[:, 1:2])
            else:
                nc.vector.tensor_add(out=mean, in0=stc[:, 0:1],
                                     in1=stc[:, 1:2])
            msq = wk.tile([128, 1], F32, tag=f"msq{tagp}", bufs=2, name="msq")
            nc.vector.tensor_add(
                out=msq, in0=stc[:, nsum:nsum + 1],
                in1=stc[:, nsum + 1:nsum + 2])
            m2 = wk.tile([128, 1], F32, tag=f"m2{tagp}", bufs=2, name="m2")
            nc.vector.tensor_mul(out=m2, in0=mean, in1=mean)
            var = wk.tile([128, 1], F32, tag=f"var{tagp}", bufs=2, name="var")
            nc.vector.scalar_tensor_tensor(
                out=var, in0=msq, scalar=EPS, in1=m2,
                op0=ALU.add, op1=ALU.subtract)
            y = wk.tile([128, 1], F32, tag=f"y{tagp}", bufs=2, name="y")
            nc.vector.reciprocal_approx_fast(out=y, in_=var)
            for _ in range(3):
                u = wk.tile([128, 1], F32, tag=f"u{tagp}", bufs=2, name="u")
                nc.vector.tensor_mul(out=u, in0=y, in1=y)
                a = wk.tile([128, 1], F32, tag=f"a{tagp}", bufs=2, name="a")
                nc.vector.tensor_mul(out=a, in0=u, in1=var)
                b = wk.tile([128, 1], F32, tag=f"b{tagp}", bufs=2, name="b")
                nc.vector.tensor_scalar(
                    out=b, in0=a, scalar1=-0.5, scalar2=1.5,
                    op0=ALU.mult, op1=ALU.add)
                y2 = wk.tile([128, 1], F32, tag=f"y{tagp}", bufs=2, name="y2")
                nc.vector.tensor_mul(out=y2, in0=y, in1=b)
                y = y2
            return mean, y

        # ---- P1a: pad + LPU (emitted inline or as fill thunks) ----
        def P1a_thunks(s, st):
            st["x1l"] = [None, None]
            st["xbl"] = [None, None]
            st["st4l1"] = None

            def build_xb():
                for ch in range(2):
                    xb = wk.tile([128, 30, 30], F8, tag=f"xb{ch}", bufs=2,
                                 name="xb")
                    if s < 2:
                        nc.vector.memset(xb, 0.0)
                    nc.vector.tensor_copy(
                        out=xb[:, 1:29, 1:29],
                        in_=xst[s][ch].rearrange("p (h w) -> p h w", w=W))
                    st["xbl"][ch] = xb

            def lpu(ch):
                if st["st4l1"] is None:
                    st["st4l1"] = wk.tile(
                        [128, 6], F32, tag="st4l1", bufs=2, name="st4")
                st["x1l"][ch] = wk.tile(
                    [128, N], BF16, tag=f"x1{ch}", bufs=3, name="x1")
                x1 = st["x1l"][ch]
                pl = [psS.tile([128, 392], F32, tag="small", name="pl")
                      for _ in range(2)]
                xb = st["xbl"][ch]
                base = ch * 1152
                for dx in range(3):
                    lhs = dgl_sb[:, base + dx * 256:base + (dx + 1) * 256].rearrange(
                        "p (t m) -> p t m", t=2)
                    for hf in range(2):
                        r0 = 14 * hf
                        nc.tensor.matmul(
                            pl[hf], lhs,
                            strided(xb[:, r0:r0 + 15, dx:dx + 28],
                                    [[900, 128], [30, 2], [30, 14], [1, 28]]),
                            start=(dx == 0), stop=False, perf_mode=DR)
                lhs = dgl_sb[:, base + 768:base + 1024].rearrange(
                    "p (t m) -> p t m", t=2)
                for hf in range(2):
                    r0 = 14 * hf
                    nc.tensor.matmul(
                        pl[hf], lhs,
                        strided(xb[:, r0 + 2:r0 + 16, 0:28],
                                [[900, 128], [1, 2], [30, 14], [1, 28]]),
                        start=False, stop=False, perf_mode=DR)
                for hf in range(2):
                    r0 = 14 * hf
                    nc.tensor.matmul(
                        pl[hf], dgl_sb[:, base + 1024:base + 1152],
                        xb[:, r0 + 2:r0 + 16, 2:30],
                        start=False, stop=True)
                for hf in range(2):
                    nc.vector.scalar_tensor_tensor(
                        out=x1[:, hf * 392:(hf + 1) * 392], in0=pl[hf],
                        scalar=lpub[ch],
                        in1=xst[s][ch][:, hf * 392:(hf + 1) * 392],
                        op0=ALU.add, op1=ALU.add,
                        accum_out=st["st4l1"][:, 2 * ch + hf:2 * ch + hf + 1])

            return [build_xb] + [
                (lambda ch=ch: lpu(ch)) for ch in range(2)]

        def P1a(s, st):
            for t in P1a_thunks(s, st):
                t()

        # ---- P1b: LN1 + kv + q/k/v projections ----
        def P1b(s, st, fq):
            x1l = st["x1l"]
            fq.take(8)

            mean1, rstd1 = ln_stats(x1l, st["st4l1"], 4, "l1")
            ln1b = []
            for ch in range(2):
                t = wk.tile([128, N], BF16, tag=f"ln1b{ch}", bufs=2,
                            name="ln1b")
                nc.vector.tensor_scalar(
                    out=t, in0=x1l[ch], scalar1=mean1, scalar2=rstd1,
                    op0=ALU.subtract, op1=ALU.mult)
                ln1b.append(t)

            kvb = []
            for ch in range(2):
                x5 = x1l[ch].rearrange(
                    "p (h a w b) -> p h a w b", h=14, a=2, w=14, b=2)
                pk = psS.tile([128, NK], F32, tag="small", name="pk")
                for t4 in range(4):
                    dy, dx = t4 // 2, t4 % 2
                    nc.tensor.matmul(
                        pk,
                        dgk_sb[:, (ch * 4 + t4) * 128:(ch * 4 + t4 + 1) * 128],
                        x5[:, :, dy, :, dx],
                        start=(t4 == 0), stop=(t4 == 3))
                t = wk.tile([128, NK], BF16, tag=f"kvb{ch}", bufs=2, name="kvb")
                nc.vector.tensor_scalar(
                    out=t, in0=pk, scalar1=dwb[ch], scalar2=None, op0=ALU.add)
                kvb.append(t)

            qb = []
            for mc in range(2):
                pq = psB.tile([128, N], F32, tag="big", name="pq")
                for kc in range(2):
                    for i0, iw in ISL:
                        nc.tensor.matmul(
                            pq[:, i0:i0 + iw],
                            wsb[:, kc * 256 + mc * 128:kc * 256 + mc * 128 + 128],
                            ln1b[kc][:, i0:i0 + iw],
                            start=(kc == 0), stop=(kc == 1))
                t = wk.tile([128, N], BF16, tag=f"qb{mc}", bufs=2, name="qb")
                nc.vector.tensor_scalar(
                    out=t, in0=pq, scalar1=bqc[mc], scalar2=None, op0=ALU.add)
                qb.append(t)
            kb = []
            for mc in range(2):
                pk2 = psS.tile([128, NK], F32, tag="small", name="pk2")
                for kc in range(2):
                    nc.tensor.matmul(
                        pk2,
                        wsb[:, 512 + kc * 256 + mc * 128:512 + kc * 256 + mc * 128 + 128],
                        kvb[kc],
                        start=(kc == 0), stop=(kc == 1))
                t = wk.tile([128, NK], BF16, tag=f"kb{mc}", bufs=2, name="kb")
                nc.vector.tensor_scalar(
                    out=t, in0=pk2, scalar1=bkc[mc], scalar2=None, op0=ALU.add)
                kb.append(t)
            vbp = wk.tile([128, 2, C], F8, tag="vbp", bufs=2, name="vbp")
            nc.gpsimd.memset(vbp[64:128, 1, :], 0.0)
            for pi, (j0, jw) in enumerate([(0, 128), (128, 68)]):
                pv = psS.tile([128, C], F32, tag="small", name="pv")
                nc.tensor.matmul(
                    pv[0:jw, :], ones1[0:1, 0:jw], bv_r, start=True, stop=False)
                for kc in range(2):
                    nc.tensor.matmul(
                        pv[0:jw, :], kvb[kc][:, j0:j0 + jw],
                        wsb[:, 1024 + kc * 256:1024 + (kc + 1) * 256],
                        start=False, stop=(kc == 1))
                nc.vector.tensor_copy(out=vbp[0:jw, pi, :], in_=pv[0:jw, :])
            st["qb"], st["kb"], st["vb"] = qb, kb, vbp

        # ---- P2: attention + out-proj + reinterpret bounce ----
        # fill: optional per-tc4 callables emitting dense PE work (previous
        # sample's c2) between the exp-paced QK stages, to keep PE duty high
        # enough that the HAM clock gate stays open.
        def P2(s, st, fq):
            qb, kb, vb = st["qb"], st["kb"], st["vb"]
            tnb = []
            for tc4 in range(2):
                pa = []
                for q4 in range(4):
                    h = tc4 * 4 + q4
                    ro = 32 * q4
                    attA = psB.tile([128, N], F32, tag="big", name="attA")
                    attB = psB.tile([128, N], F32, tag="big", name="attB")
                    for i0, iw in ISL:
                        nc.tensor.matmul(
                            attA[:, i0:i0 + iw], kb[tc4][ro:ro + 32, 0:128],
                            qb[tc4][ro:ro + 32, i0:i0 + iw], start=True,
                            stop=True, tile_position=(ro, 0))
                    for i0, iw in ISL:
                        nc.tensor.matmul(
                            attB[0:68, i0:i0 + iw], kb[tc4][ro:ro + 32, 128:NK],
                            qb[tc4][ro:ro + 32, i0:i0 + iw], start=True,
                            stop=True, tile_position=(ro, 0))
                    pq4 = wk.tile([128, 2, N], F8, tag=f"pa{q4}", bufs=3,
                                  name="pq4")
                    if s < 2:
                        # B-plane rows >=68 are never written but ARE read
                        # (x0 mask) by the DoubleRow sum matmul; scrub each
                        # slot's initial garbage (could be fp8 NaN patterns);
                        # bufs=3 so the third slot first appears in sample 1
                        nc.gpsimd.memset(pq4[64:128, 1, :], 0.0)
                    pe1 = wk.tile([128, N], BF16, tag="pexp", bufs=4,
                                  name="pe1")
                    nc.scalar.activation(out=pe1, in_=attA, func=AF.Exp)
                    nc.vector.tensor_mul(
                        out=pq4[:, 0, :], in0=pe1,
                        in1=e8[:, h * 1568:h * 1568 + N])
                    pe2 = wk.tile([128, N], BF16, tag="pexp", bufs=4,
                                  name="pe2")
                    nc.scalar.activation(
                        out=pe2[0:68, :], in_=attB[0:68, :], func=AF.Exp)
                    nc.vector.tensor_mul(
                        out=pq4[0:68, 1, :], in0=pe2[0:68, :],
                        in1=e8[0:68, h * 1568 + N:h * 1568 + 2 * N])
                    pa.append(pq4)

                fq.take(2)

                S_ps = psB.tile([128, N], F32, tag="big", name="S_ps")
                for i0, iw in ISL:
                    for q4 in range(4):
                        nc.tensor.matmul(
                            S_ps[:, i0:i0 + iw],
                            bh[:, q4 * 256:(q4 + 1) * 256].rearrange(
                                "p (t m) -> p t m", t=2),
                            pa[q4][:, :, i0:i0 + iw], start=(q4 == 0),
                            stop=(q4 == 3), perf_mode=DR)
                rS = wk.tile([128, N], F32, tag=f"rS{tc4}", bufs=1, name="rS")
                nc.vector.reciprocal_approx_fast(out=rS, in_=S_ps)

                tun = psB.tile([128, N], F32, tag="big", name="tun")
                for q4 in range(4):
                    h = tc4 * 4 + q4
                    ro = 32 * q4
                    for i0, iw in ISL:
                        nc.tensor.matmul(
                            tun[ro:ro + 32, i0:i0 + iw],
                            vb[:, 0, 32 * h:32 * h + 32],
                            pa[q4][:, 0, i0:i0 + iw], start=True, stop=False,
                            tile_position=(0, ro))
                    for i0, iw in ISL:
                        nc.tensor.matmul(
                            tun[ro:ro + 32, i0:i0 + iw],
                            vb[0:68, 1, 32 * h:32 * h + 32],
                            pa[q4][0:68, 1, i0:i0 + iw], start=False,
                            stop=True, tile_position=(0, ro))
                t = wk.tile([128, N], BF16, tag=f"tnb{tc4}", bufs=2, name="tnb")
                nc.vector.tensor_mul(out=t, in0=tun, in1=rS)
                tnb.append(t)

            # out-proj + DRAM bounce (raw reinterpret), ch-half at a time
            orel = [wk.tile([128, N], BF16, tag=f"ore{ch}", bufs=1, name="ore")
                    for ch in range(2)]
            for j in range(8):
                n0 = j * 98
                po = psS.tile([128, C], F32, tag="small", name="po")
                for tc4 in range(2):
                    nc.tensor.matmul(
                        po[0:98, :], tnb[tc4][:, n0:n0 + 98],
                        wsb[:, 1536 + tc4 * 256:1536 + (tc4 + 1) * 256],
                        start=(tc4 == 0), stop=(tc4 == 1))
                osb = wk.tile([128, C], BF16, tag="osb", bufs=2, name="osb")
                nc.vector.tensor_copy(out=osb[0:98, :], in_=po[0:98, :])
                # raw row-major reinterpret via DRAM bounce, both legs on the
                # scalar ring (kept free of bulk traffic)
                nc.sync.dma_start(
                    out=scr_d[s, n0 * C:(n0 + 98) * C].rearrange(
                        "(n c) -> n c", c=C),
                    in_=osb[0:98, :])
                nc.scalar.dma_start(
                    out=orel[j // 4][32 * (j % 4):32 * (j % 4) + 32, :],
                    in_=scr_d[s, j * 25088:(j + 1) * 25088].rearrange(
                        "(a i) -> a i", i=N))
                if j == 3:
                    fq.take(1)
            st["orel"] = orel

        # ---- P3: residual (+bo) + LN2 ----
        def P3(s, st, fq):
            x1l, orel = st["x1l"], st["orel"]
            st4b = wk.tile([128, 4], F32, tag="st4l2", bufs=2, name="st4b")
            x2l = []
            for ch in range(2):
                t = wk.tile([128, N], BF16, tag=f"x2{ch}", bufs=2, name="x2")
                nc.vector.scalar_tensor_tensor(
                    out=t, in0=orel[ch], scalar=boc[ch], in1=x1l[ch],
                    op0=ALU.add, op1=ALU.add,
                    accum_out=st4b[:, ch:ch + 1])
                x2l.append(t)
            fq.take(1)
            mean2, rstd2 = ln_stats(x2l, st4b, 2, "l2")
            ln2p = wk.tile([128, 2, N], F8, tag="ln2p", bufs=2, name="ln2p")
            for ch in range(2):
                nc.vector.tensor_scalar(
                    out=ln2p[:, ch, :], in0=x2l[ch], scalar1=mean2,
                    scalar2=rstd2, op0=ALU.subtract, op1=ALU.mult)
            st["x2l"], st["ln2b"] = x2l, ln2p
            fq.take(1)

        # ---- P4a: IRFFN expand (c1) ----
        def P4a(s, st):
            ln2p = st["ln2b"]
            h1p = []
            for mc in range(8):
                pc1 = psB.tile([128, N], F32, tag="big", name="pc1")
                for i0, iw in ISL:
                    nc.tensor.matmul(
                        pc1[:, i0:i0 + iw],
                        c1t[:, mc * 256:mc * 256 + 256].rearrange(
                            "p (t m) -> p t m", t=2),
                        ln2p[:, :, i0:i0 + iw],
                        start=True, stop=True, perf_mode=DR)
                hp = wk.tile([128, 30, 30], F8, tag=f"h1p{mc}", bufs=1,
                             name="hp")
                if s == 0:
                    nc.vector.memset(hp, 0.0)
                nc.scalar.activation(
                    out=hp[:, 1:29, 1:29],
                    in_=pc1.rearrange("p (h w) -> p h w", w=W),
                    func=AF.Gelu, scale=A1c[mc], bias=B1c[mc])
                h1p.append(hp)
            st["h1p"] = h1p
            st["h2p"] = [None] * 4

        # ---- dw2: one 128-channel depthwise chunk (both halves); taps
        # paired into fp8 DoubleRow matmuls via overlapping-stride APs ----
        def dw2_chunk(s, st, mc):
            hp = st["h1p"][mc]
            if st["h2p"][mc // 2] is None:
                st["h2p"][mc // 2] = wk.tile(
                    [128, 2, N], F8, tag=f"h2p{mc // 2}", bufs=1, name="h2p")
            t = st["h2p"][mc // 2]
            base = mc * 1152
            pd = [psS.tile([128, 392], F32, tag="small", name="pd")
                  for _ in range(2)]
            for dx in range(3):
                lhs = dgd_sb[:, base + dx * 256:base + (dx + 1) * 256].rearrange(
                    "p (t m) -> p t m", t=2)
                for hf in range(2):
                    r0 = 14 * hf
                    nc.tensor.matmul(
                        pd[hf], lhs,
                        strided(hp[:, r0:r0 + 15, dx:dx + 28],
                                [[900, 128], [30, 2], [30, 14], [1, 28]]),
                        start=(dx == 0), stop=False, perf_mode=DR)
            lhs = dgd_sb[:, base + 768:base + 1024].rearrange(
                "p (t m) -> p t m", t=2)
            for hf in range(2):
                r0 = 14 * hf
                nc.tensor.matmul(
                    pd[hf], lhs,
                    strided(hp[:, r0 + 2:r0 + 16, 0:28],
                            [[900, 128], [1, 2], [30, 14], [1, 28]]),
                    start=False, stop=False, perf_mode=DR)
            for hf in range(2):
                r0 = 14 * hf
                nc.tensor.matmul(
                    pd[hf], dgd_sb[:, base + 1024:base + 1152],
                    hp[:, r0 + 2:r0 + 16, 2:30],
                    start=False, stop=True)
            for hf in range(2):
                nc.scalar.activation(
                    out=t[:, mc % 2, hf * 392:(hf + 1) * 392], in_=pd[hf],
                    func=AF.Gelu, scale=A2c[mc], bias=B2c[mc])

        # ---- P4b: IRFFN contract (c2 + residual + store), one 128-ch
        # chunk; emitted as PE fill inside the next sample's P2 ----
        def P4b_chunk(s, st, mc):
            x2l, h2p = st["x2l"], st["h2p"]
            pc2 = psB.tile([128, N], F32, tag="big", name="pc2")
            for p in range(4):
                for i0, iw in ISL:
                    nc.tensor.matmul(
                        pc2[:, i0:i0 + iw],
                        c2t[:, p * 512 + mc * 256:p * 512 + mc * 256 + 256].rearrange(
                            "p (t m) -> p t m", t=2),
                        h2p[p][:, :, i0:i0 + iw],
                        start=(p == 0), stop=(p == 3), perf_mode=DR)
            t3 = wk.tile([128, N], F32, tag="t3", bufs=2, name="t3")
            nc.scalar.activation(
                out=t3, in_=pc2, func=AF.Identity, scale=A3c[mc],
                bias=B3c[mc])
            yt = wk.tile([128, N], F32, tag="yt", bufs=2, name="yt")
            nc.vector.tensor_add(out=yt, in0=t3, in1=x2l[mc])
            nc.gpsimd.dma_start(
                out=yv[s, mc * 128:(mc + 1) * 128, :], in_=yt)

        # ---- software-pipelined emission across samples ----
        class FQ:
            def __init__(self):
                self.q = []

            def add(self, *thunks):
                self.q.extend(thunks)

            def take(self, n):
                for _ in range(min(n, len(self.q))):
                    self.q.pop(0)()

            def drain(self):
                self.take(len(self.q))

        def ffn_tail(s, st):
            return ([lambda mc=mc: dw2_chunk(s, st, mc) for mc in range(8)]
                    + [lambda: P4b_chunk(s, st, 0), lambda: P4b_chunk(s, st, 1)])

        fq = FQ()
        st = [dict() for _ in range(S)]
        P1a(0, st[0])
        P1a(1, st[1])
        P1b(0, st[0], fq)
        P1b(1, st[1], fq)
        fq.add(*P1a_thunks(2, st[2]))
        P2(0, st[0], fq)
        P3(0, st[0], fq)
        fq.add(*P1a_thunks(3, st[3]))
        P2(1, st[1], fq)
        P4a(0, st[0])
        fq.add(*ffn_tail(0, st[0]))
        P1b(2, st[2], fq)
        P3(1, st[1], fq)
        P2(2, st[2], fq)
        fq.drain()
        P4a(1, st[1])
        fq.add(*ffn_tail(1, st[1]))
        P1b(3, st[3], fq)
        P3(2, st[2], fq)
        P2(3, st[3], fq)
        fq.drain()
        P4a(2, st[2])
        fq.add(*ffn_tail(2, st[2]))
        P3(3, st[3], fq)
        fq.drain()
        P4a(3, st[3])
        fq.add(*ffn_tail(3, st[3]))
        fq.drain()

    nc.finalize()
    _CACHE["nc"] = nc
    return nc


def _prep_shared(inputs):
    """Host-side packing of all weights into upload-ready layouts."""
    f = np.float32
    g = {k: np.asarray(v, f) for k, v in inputs.items()}

    def wT(w):
        # w [M, K] -> [128, (K//128)*M], cols kc*M + m
        K, M = w.shape[1], w.shape[0]
        t = np.ascontiguousarray(w.T).reshape(K // 128, 128, M)
        return np.concatenate(list(t), axis=1)

    wq_s = g["wq"] * SCALE
    wpk = np.concatenate(
        [wT(wq_s), wT(g["wk"]), wT(g["wv"]), wT(g["wo"])], axis=1).astype(BFNP)
    # c1: cols mc*256 + kc*128 + m (DoubleRow kc pairs adjacent)
    c1w = g["c1_w"].reshape(CM, C)  # [M=1024, K=256]
    c1t = np.zeros((128, 2048), np.float32)
    for mc in range(8):
        for kc in range(2):
            c1t[:, mc * 256 + kc * 128:mc * 256 + (kc + 1) * 128] = \
                c1w[mc * 128:(mc + 1) * 128, kc * 128:(kc + 1) * 128].T
    c1t = c1t.astype(F8NP)
    # c2: cols p*512 + mc*256 + t*128 + m (kc pairs 2p,2p+1)
    c2w = g["c2_w"].reshape(C, CM)  # [M=256, K=1024]
    c2t = np.zeros((128, 2048), np.float32)
    for p in range(4):
        for mc in range(2):
            for t in range(2):
                kc = 2 * p + t
                c2t[:, p * 512 + mc * 256 + t * 128:
                    p * 512 + mc * 256 + (t + 1) * 128] = \
                    c2w[mc * 128:(mc + 1) * 128, kc * 128:(kc + 1) * 128].T
    c2t = c2t.astype(F8NP)

    e = np.exp(g["pos_b"][0])            # [8, 784, 196]
    eT = e.transpose(0, 2, 1)            # [8, 196, 784]
    E = np.zeros((8, 128, 2 * N), f)
    E[:, :, :N] = eT[:, :128, :]
    E[:, :68, N:] = eT[:, 128:, :]
    e8 = np.concatenate(list(E), axis=1).astype(BFNP)

    def diag_pack(w, taps):
        Cn = w.shape[0]
        wf = w.reshape(Cn, taps)
        cols = []
        for gi in range(Cn // 128):
            for t in range(taps):
                d = np.zeros((128, 128), f)
                np.fill_diagonal(d, wf[gi * 128:(gi + 1) * 128, t])
                cols.append(d)
        return np.concatenate(cols, axis=1).astype(BFNP)

    def diag_pack_pairs(w, Cn):
        wf = w.reshape(Cn, 9)
        cols = []
        for gi in range(Cn // 128):
            sl = wf[gi * 128:(gi + 1) * 128]

            def diag(t9):
                dd = np.zeros((128, 128), np.float32)
                np.fill_diagonal(dd, sl[:, t9])
                return dd

            for dx in range(3):
                cols += [diag(0 + dx), diag(3 + dx)]
            cols += [diag(6), diag(7), diag(8)]
        return np.concatenate(cols, axis=1).astype(F8NP)

    dgl = diag_pack_pairs(g["lpu_w"], C)
    dgk = diag_pack(g["dw_w"], 4)
    # dw2 diags in DoubleRow pair layout, fp8:
    # per mc (1152 cols): [d(0,dx)|d(1,dx)] for dx=0..2, [d(2,0)|d(2,1)], d(2,2)
    wf2 = g["dw2_w"].reshape(CM, 9)
    cols = []
    for mc in range(8):
        sl = wf2[mc * 128:(mc + 1) * 128]

        def diag(t9):
            dd = np.zeros((128, 128), np.float32)
            np.fill_diagonal(dd, sl[:, t9])
            return dd

        for dx in range(3):
            cols += [diag(0 + dx), diag(3 + dx)]
        cols += [diag(6), diag(7), diag(8)]
    dgd = np.concatenate(cols, axis=1).astype(F8NP)

    bc = np.zeros((128, 48), f)
    bq_s = g["bq"] * SCALE
    for i, v in enumerate([g["lpu_b"], g["dw_b"], bq_s, g["bk"], g["bo"]]):
        bc[:, 2 * i] = v[:128]
        bc[:, 2 * i + 1] = v[128:]
    bc[:, 10] = EPS
    A1 = g["bn1_g"] / np.sqrt(g["bn1_v"] + EPS)
    B1 = g["bn1_b"] - g["bn1_m"] * A1 + A1 * g["c1_b"]
    A2 = g["bn2_g"] / np.sqrt(g["bn2_v"] + EPS)
    B2 = g["bn2_b"] - g["bn2_m"] * A2 + A2 * g["dw2_b"]
    A3 = g["bn3_g"] / np.sqrt(g["bn3_v"] + EPS)
    B3 = g["bn3_b"] - g["bn3_m"] * A3 + A3 * g["c2_b"]
    bc[:, 12:20] = A1.reshape(8, 128).T
    bc[:, 20:28] = B1.reshape(8, 128).T
    bc[:, 28:36] = A2.reshape(8, 128).T
    bc[:, 36:44] = B2.reshape(8, 128).T
    bc[:, 44:46] = A3.reshape(2, 128).T
    bc[:, 46:48] = B3.reshape(2, 128).T

    br = np.zeros((1, 384), f)
    br[0, :256] = g["bv"]
    br[0, 256:] = 1.0

    # per q4: [0:128) = A mask (all 128 j rows), [128:256) = B mask (j<68)
    bh = np.zeros((128, 1024), f)
    for q in range(4):
        bh[:, q * 256 + 32 * q:q * 256 + 32 * q + 32] = 1.0
        bh[0:68, q * 256 + 128 + 32 * q:q * 256 + 128 + 32 * q + 32] = 1.0

    idn = np.eye(128, dtype=np.float32)
    return dict(
        wpk=wpk, c1t=c1t, c2t=c2t, e8=e8, dgl=dgl, dgk=dgk, dgd=dgd,
        bcol=np.ascontiguousarray(bc),
        brow=br.astype(BFNP), bh4=bh.astype(F8NP), idn=idn.astype(BFNP))


def _in_maps(inputs):
    shared = _prep_shared(inputs)
    x = np.asarray(inputs["x"], np.float32).astype(BFNP)
    maps = []
    for c in range(NCORES):
        m = dict(shared)
        m["x"] = np.ascontiguousarray(x[c * S:(c + 1) * S])
        maps.append(m)
    return maps


def kernel(**inputs):
    nc = _build()
    res = run_bass_kernel_spmd(nc, _in_maps(inputs),
                               core_ids=list(range(NCORES)))
    out = np.concatenate([res.results[c]["y"] for c in range(NCORES)], axis=0)
    return out
